# revision 18
# baseline (speedup 1.0000x reference)
"""Trainium2 Bass kernel for AdvancedConvBlock: conv3x3 + batch-stat LN + RoPE
attention with ALiBi + proj + residual, data-parallel over batch on 8 cores.

Self-contained: hardcodes shapes B=8, C=128, H=W=32, heads=8, d=16.
"""

import sys

sys.path.insert(0, "/opt/trn_rl_repo")

import numpy as np
from contextlib import ExitStack

import concourse.bass as bass
import concourse.tile as tile
from concourse import mybir
from concourse import bacc
from concourse.bass_utils import run_bass_kernel_spmd

F32 = mybir.dt.float32
BF16 = mybir.dt.bfloat16
NPBF16 = mybir.dt.np(mybir.dt.bfloat16)

NCORES = 8
C = 128
H = W = 32
N = H * W  # 1024 tokens
NHEADS = 8
D = 16  # head dim
SCALE = D ** (-0.5)
ALIBI_MAX_BIAS = 8.0
EPS = 1e-5
TOTAL = NCORES * N  # stat count 8192

AX = mybir.AxisListType
ALU = mybir.AluOpType
ACT = mybir.ActivationFunctionType


def _alibi_slopes(n: int) -> np.ndarray:
    start = 2.0 ** (-(2.0 ** (-(np.log2(n) - 3.0))))
    return np.array([start * (start ** i) for i in range(n)], dtype=np.float32)


SLOPE8 = _alibi_slopes(NHEADS) * ALIBI_MAX_BIAS  # per-head bias multiplier


# ---------------------------------------------------------------- kernel build
def build_kernel(tc: tile.TileContext, io: dict, stage: int = 99):
    nc = tc.nc
    ctx = ExitStack()
    sb = ctx.enter_context(tc.tile_pool(name="sb", bufs=1))
    work = ctx.enter_context(tc.tile_pool(name="work", bufs=3))
    epool = ctx.enter_context(tc.tile_pool(name="e", bufs=6))
    ps = ctx.enter_context(tc.tile_pool(name="ps", bufs=1, space="PSUM"))
    av_pool = ctx.enter_context(tc.tile_pool(name="av", bufs=2, space="PSUM"))
    dram = ctx.enter_context(tc.tile_pool(name="dram", bufs=1, space="DRAM"))

    # ---- persistent inputs to SBUF (need-order; big table last, own queue)
    x_f32 = sb.tile([128, N], F32)
    nc.sync.dma_start(out=x_f32, in_=io["xs"])
    cw = sb.tile([128, 9, 128], BF16)
    nc.sync.dma_start(out=cw, in_=io["cwT"])
    cb = sb.tile([128, 1], F32)
    nc.sync.dma_start(out=cb, in_=io["cb"])
    qwA = sb.tile([128, 128], BF16)
    nc.sync.dma_start(out=qwA, in_=io["qwA"])
    qwB = sb.tile([128, 128], BF16)
    nc.sync.dma_start(out=qwB, in_=io["qwB"])
    kwA = sb.tile([128, 128], BF16)
    nc.sync.dma_start(out=kwA, in_=io["kwA"])
    kwB = sb.tile([128, 128], BF16)
    nc.sync.dma_start(out=kwB, in_=io["kwB"])
    qwAr = sb.tile([128, 128], BF16)
    nc.sync.dma_start(out=qwAr, in_=io["qwAr"])
    qwBr = sb.tile([128, 128], BF16)
    nc.sync.dma_start(out=qwBr, in_=io["qwBr"])
    kwAr = sb.tile([128, 128], BF16)
    nc.sync.dma_start(out=kwAr, in_=io["kwAr"])
    kwBr = sb.tile([128, 128], BF16)
    nc.sync.dma_start(out=kwBr, in_=io["kwBr"])
    vw = sb.tile([128, 256], BF16)
    nc.sync.dma_start(out=vw, in_=io["vw"])
    cos = sb.tile([128, N], F32)
    nc.sync.dma_start(out=cos, in_=io["cos"])
    sin = sb.tile([128, N], F32)
    nc.sync.dma_start(out=sin, in_=io["sin"])
    pwA = sb.tile([128, 128], BF16)
    nc.sync.dma_start(out=pwA, in_=io["pwA"])
    pwB = sb.tile([128, 128], BF16)
    nc.sync.dma_start(out=pwB, in_=io["pwB"])
    pb = sb.tile([128, 1], F32)
    nc.sync.dma_start(out=pb, in_=io["pb"])
    m_sb = sb.tile([128, 8, 1536], BF16)  # alibi decay table per head
    nc.gpsimd.dma_start(out=m_sb, in_=io["m"])

    # ---- conv 3x3 pad 1 on ALL 8 images (own image = slot 0); global batch
    # stats computed locally — no collective needed.
    xbs = []
    for img in range(8):
        xb = sb.tile([128, N], BF16, tag=f"xb{img}", name=f"xb{img}")
        nc.sync.dma_start(out=xb, in_=io["xall"][img])
        xbs.append(xb)
    scol = sb.tile([128, 8], F32)
    sqcol = sb.tile([128, 8], F32)
    y_sb = sb.tile([128, N], F32)
    for img in range(8):
        xpad = work.tile([128, 34, 34], BF16, tag="xpad")
        nc.vector.memset(xpad, 0.0)
        nc.vector.tensor_copy(
            xpad[:, 1:33, 1:33], xbs[img].rearrange("p (h w) -> p h w", h=H)
        )
        conv_ps = ps.tile([128, N], F32, tag="ps")
        conv_v = conv_ps.rearrange("p (h w) -> p h w", h=H)
        for t in range(9):
            dh, dw = t // 3, t % 3
            for hc in range(2):
                nc.tensor.matmul(
                    out=conv_v[:, hc * 16 : hc * 16 + 16, :],
                    lhsT=cw[:, t, :],
                    rhs=xpad[:, dh + hc * 16 : dh + hc * 16 + 16, dw : dw + 32],
                    start=(t == 0),
                    stop=(t == 8),
                )
        nc.vector.tensor_reduce(
            scol[:, img : img + 1], conv_ps, axis=AX.X, op=ALU.add
        )
        sq_dump = work.tile([128, N], F32, tag="sqd")
        nc.scalar.activation(
            sq_dump, conv_ps, ACT.Square, accum_out=sqcol[:, img : img + 1]
        )
        if img == 0:
            nc.vector.tensor_scalar_add(y_sb, conv_ps, cb)

    # global per-channel stats of y = conv + cb over all 8*1024 samples
    s_t = sb.tile([128, 1], F32)
    nc.vector.tensor_reduce(s_t, scol, axis=AX.X, op=ALU.add)
    sq_t = sb.tile([128, 1], F32)
    nc.vector.tensor_reduce(sq_t, sqcol, axis=AX.X, op=ALU.add)
    mean0 = sb.tile([128, 1], F32)
    nc.scalar.mul(mean0, s_t, 1.0 / TOTAL)
    mean = sb.tile([128, 1], F32)
    nc.vector.tensor_add(mean, mean0, cb)
    ex2 = sb.tile([128, 1], F32)
    nc.scalar.mul(ex2, sq_t, 1.0 / TOTAL)
    # ex2 of (conv+cb) = E[conv^2] + cb*(2*mean0 + cb)
    t2m = sb.tile([128, 1], F32)
    nc.vector.tensor_add(t2m, mean0, mean0)
    nc.vector.tensor_add(t2m, t2m, cb)
    nc.vector.tensor_mul(t2m, t2m, cb)
    nc.vector.tensor_add(ex2, ex2, t2m)
    var = sb.tile([128, 1], F32)
    nc.vector.tensor_mul(var, mean, mean)
    nc.vector.tensor_sub(var, ex2, var)
    eps_t = sb.tile([128, 1], F32)
    nc.vector.memset(eps_t, EPS)
    std = sb.tile([128, 1], F32)
    nc.scalar.activation(std, var, ACT.Sqrt, bias=eps_t)
    rstd = sb.tile([128, 1], F32)
    nc.vector.reciprocal(rstd, std)
    nmb = sb.tile([128, 1], F32)
    nc.vector.tensor_mul(nmb, mean, rstd)
    nc.scalar.mul(nmb, nmb, -1.0)
    y_n = sb.tile([128, N], BF16)
    nc.scalar.activation(y_n, y_sb, ACT.Identity, bias=nmb, scale=rstd)
    if stage <= 1:
        dbg = sb.tile([128, N], F32)
        nc.vector.tensor_copy(dbg, y_n)
        nc.sync.dma_start(out=io["out"], in_=dbg)
        ctx.close()
        return

    # ---- qkv with RoPE fused: q' = (W y)*cos + ((P W) y)*sin, packed heads
    def qk_rope(wt, wrt, name):
        p0 = ps.tile([128, N], F32, tag="ps")
        p1 = ps.tile([128, N], F32, tag="ps")
        for c in range(2):
            sl = slice(c * 512, (c + 1) * 512)
            nc.tensor.matmul(
                out=p0[:, sl], lhsT=wt, rhs=y_n[:, sl], start=True, stop=True
            )
            nc.tensor.matmul(
                out=p1[:, sl], lhsT=wrt, rhs=y_n[:, sl], start=True, stop=True
            )
        t1 = work.tile([128, N], F32, tag="ropet1")
        nc.vector.tensor_mul(t1, p0, cos)
        t2 = work.tile([128, N], F32, tag="ropet2")
        nc.vector.tensor_mul(t2, p1, sin)
        out = sb.tile([128, N], BF16, tag=name)
        nc.vector.tensor_add(out, t1, t2)
        return out

    qAr = qk_rope(qwA, qwAr, "qAr")
    qBr = qk_rope(qwB, qwBr, "qBr")
    kAr = qk_rope(kwA, kwAr, "kAr")
    kBr = qk_rope(kwB, kwBr, "kBr")

    # ---- v transposed: vt[j, head, dcol] with a ones column at dcol=16
    vt = sb.tile([128, 8, 8, 32], BF16)  # [j-part, jc, head, 32]
    for jc in range(8):
        vp = ps.tile([128, 256], F32, tag="ps")
        nc.tensor.matmul(
            out=vp,
            lhsT=y_n[:, jc * 128 : (jc + 1) * 128],
            rhs=vw,
            start=True,
            stop=True,
        )
        nc.vector.tensor_copy(vt[:, jc], vp.rearrange("p (h c) -> p h c", c=32))
    nc.vector.memset(vt[:, :, :, 0:1], 1.0)

    if stage <= 2:
        dbg = sb.tile([128, N], F32)
        nc.vector.tensor_copy(dbg, qAr)
        nc.vector.tensor_add(dbg, dbg, kBr)
        nc.sync.dma_start(out=io["out"], in_=dbg)
        ctx.close()
        return

    # ---- attention: transposed scores s[j, i], z-deferred softmax.
    # Per (group, key-chunk jc, query-half ic): 4 packed score MMs into one
    # [128, 2048] psum tile, one exp, one decay-table multiply, 4 AV MMs.
    SKIP_SET = {0: {0, 1, 2}, 1: {0, 1, 2}, 2: {0, 1, 2}, 3: {0, 1, 2},
                4: {0, 1}, 5: {0}, 6: set(), 7: set()}

    def present(h, jc, ic):
        return ic == 0 or jc not in SKIP_SET[h]

    def jc_range(h, ic):
        return [jc for jc in range(8) if present(h, jc, ic)]

    o_pks = []
    for g in range(2):
        q_r = qAr if g == 0 else qBr
        k_r = kAr if g == 0 else kBr
        o_acc = av_pool.tile([128, N], F32)
        pend = []

        def flush_av(n_keep):
            while len(pend) > n_keep:
                e2_, pres_, hp_, jc_, ic_ = pend.pop(0)
                isl_ = slice(ic_ * 512, (ic_ + 1) * 512)
                for hh in pres_:
                    h = 4 * g + hh
                    jr = jc_range(h, ic_)
                    nc.tensor.matmul(
                        out=o_acc[32 * hh : 32 * hh + 32, isl_],
                        lhsT=vt[:, jc_, h, :],
                        rhs=e2_[:, hh - 2 * hp_, :],
                        start=(jc_ == jr[0]),
                        stop=(jc_ == jr[-1]),
                        tile_position=(0, 32 * hh),
                        skip_group_check=True,
                    )

        for jc in range(8):
            for ic in range(2):
                isl = slice(ic * 512, (ic + 1) * 512)
                touches_past = 128 * jc < 512 * (ic + 1)
                off = 512 - 128 * jc + 512 * ic
                for hp in range(2):
                    pres = [
                        hh
                        for hh in (2 * hp, 2 * hp + 1)
                        if present(4 * g + hh, jc, ic)
                    ]
                    if not pres:
                        continue
                    s2 = ps.tile([128, 2, 512], F32, tag="ps")
                    for hh in pres:
                        nc.tensor.matmul(
                            out=s2[:, hh - 2 * hp, :],
                            lhsT=k_r[
                                32 * hh : 32 * hh + 16, jc * 128 : (jc + 1) * 128
                            ],
                            rhs=q_r[32 * hh : 32 * hh + 16, isl],
                            start=True,
                            stop=True,
                            tile_position=(32 * hh, 0),
                        )
                    lo = pres[0] - 2 * hp
                    e2 = epool.tile([128, 2, 512], BF16, tag="e")
                    nc.scalar.activation(e2[:, lo:, :], s2[:, lo:, :], ACT.Exp)
                    if touches_past:
                        nc.vector.tensor_mul(
                            e2[:, lo:, :],
                            e2[:, lo:, :],
                            m_sb[:, 4 * g + pres[0] : 4 * g + 2 * hp + 2, off : off + 512],
                        )
                    pend.append((e2, pres, hp, jc, ic))
                    flush_av(2)
        flush_av(0)
        # normalize: row 32h is Z (DRAM-roundtrip broadcast), rows +1..+16 are o
        o_pk = sb.tile([128, N], BF16, tag=f"opk{g}")
        zsb = work.tile([128, N], F32, tag="zsb")
        nc.vector.tensor_copy(zsb, o_acc)
        zd = dram.tile([4, N], F32, tag="zd")
        nc.sync.dma_start(out=zd, in_=zsb[0:128:32, :])
        zbc = work.tile([128, N], F32, tag="zbc")
        for hh in range(4):
            nc.sync.dma_start(
                out=zbc[32 * hh : 32 * hh + 32, :],
                in_=zd[hh : hh + 1, :].broadcast_to([32, N]),
            )
        rz = work.tile([128, N], F32, tag="rz")
        nc.vector.reciprocal_approx_fast(rz, zbc)
        nc.vector.tensor_mul(o_pk, zsb, rz)
        o_pks.append(o_pk)
    if stage <= 3:
        dbg = sb.tile([128, N], F32)
        nc.vector.tensor_copy(dbg, o_pks[0])
        nc.sync.dma_start(out=io["out"], in_=dbg)
        ctx.close()
        return

    # ---- output proj + bias + residual
    pr_ps = ps.tile([128, N], F32, tag="ps")
    for c in range(2):
        sl = slice(c * 512, (c + 1) * 512)
        nc.tensor.matmul(
            out=pr_ps[:, sl], lhsT=pwA, rhs=o_pks[0][:, sl], start=True, stop=False
        )
        nc.tensor.matmul(
            out=pr_ps[:, sl], lhsT=pwB, rhs=o_pks[1][:, sl], start=False, stop=True
        )
    out_sb = sb.tile([128, N], F32)
    nc.vector.scalar_tensor_tensor(
        out=out_sb,
        in0=pr_ps,
        scalar=pb,
        in1=x_f32,
        op0=ALU.add,
        op1=ALU.add,
    )
    nc.sync.dma_start(out=io["out"], in_=out_sb)
    ctx.close()


# ---------------------------------------------------------------- host side
def prep_host(conv_w, conv_b, qkv_w, proj_w, proj_b):
    """Precompute packed / transposed weight + table arrays shared by all cores."""
    cwT = (
        conv_w.astype(np.float32)
        .transpose(1, 2, 3, 0)
        .reshape(128, 9, 128)
        .astype(NPBF16)
    )
    qw = qkv_w[0:128]  # [128 feat, 128 ci]
    kw = qkv_w[128:256]
    vwm = qkv_w[256:384]

    def pack_qk(wm, scale):
        # lhsT[ci, 32g+r] = wm[16g+r, ci] * scale for r<16 else 0  (two halves)
        outA = np.zeros((128, 128), np.float32)
        outB = np.zeros((128, 128), np.float32)
        for g in range(4):
            for r in range(16):
                outA[:, 32 * g + r] = wm[16 * g + r, :] * scale
                outB[:, 32 * g + r] = wm[16 * (g + 4) + r, :] * scale
        return outA.astype(NPBF16), outB.astype(NPBF16)

    qwA, qwB = pack_qk(qw, SCALE)
    kwA, kwB = pack_qk(kw, 1.0)
    # rotate-half fold: rot(W y) = (P W) y, applied to packed lhsT [ci, m]
    P = np.zeros((128, 128), np.float32)
    for gg in range(4):
        b = 32 * gg
        for r in range(8):
            P[b + r, b + r + 8] = -1.0
            P[b + r + 8, b + r] = 1.0
    def rot(w):
        return (w.astype(np.float32) @ P.T).astype(NPBF16)
    qwAr, qwBr = rot(qwA), rot(qwB)
    kwAr, kwBr = rot(kwA), rot(kwB)
    vw = np.zeros((128, 256), np.float32)
    for h in range(8):
        for d in range(16):
            vw[:, 32 * h + 1 + d] = vwm[16 * h + d, :]
    vw = vw.astype(NPBF16)

    # proj: lhsT[packed_row, c'] = proj_w[c', feat(packed_row)]
    pwA = np.zeros((128, 128), np.float32)
    pwB = np.zeros((128, 128), np.float32)
    for g in range(4):
        for r in range(16):
            pwA[32 * g + 1 + r, :] = proj_w[:, 16 * g + r]
            pwB[32 * g + 1 + r, :] = proj_w[:, 16 * (g + 4) + r]
    pwA = pwA.astype(NPBF16)
    pwB = pwB.astype(NPBF16)

    inv_freq = 1.0 / (10000.0 ** (np.arange(0, D, 2, dtype=np.float32) / D))
    pos = np.arange(N, dtype=np.float32)
    freqs = pos[:, None] * inv_freq[None, :]  # [N, 8]
    cos_t = np.zeros((128, N), np.float32)
    sin_t = np.zeros((128, N), np.float32)
    for g in range(4):
        for r in range(16):
            cos_t[32 * g + r, :] = np.cos(freqs[:, r % 8])
            sin_t[32 * g + r, :] = np.sin(freqs[:, r % 8])

    # alibi decay table tblm[p, h, c] = exp(slope8[h] * min(p - c + 512, 0))
    p_ = np.arange(128, dtype=np.float64)[:, None, None]
    c_ = np.arange(1536, dtype=np.float64)[None, None, :]
    d_ = np.minimum(p_ - c_ + 512.0, 0.0)
    m = np.exp(SLOPE8.astype(np.float64)[None, :, None] * d_).astype(NPBF16)

    return dict(
        cwT=cwT,
        qwA=qwA,
        qwB=qwB,
        kwA=kwA,
        kwB=kwB,
        vw=vw,
        pwA=pwA,
        pwB=pwB,
        qwAr=qwAr,
        qwBr=qwBr,
        kwAr=kwAr,
        kwBr=kwBr,
        cos=cos_t,
        sin=sin_t,
        m=m,
        cb=conv_b.astype(np.float32).reshape(128, 1),
        pb=proj_b.astype(np.float32).reshape(128, 1),
    )


_SPECS = [
    ("xs", [128, N], F32),
    ("xall", [8, 128, N], BF16),
    ("m", [128, 8, 1536], BF16),
    ("cwT", [128, 9, 128], BF16),
    ("qwA", [128, 128], BF16),
    ("qwB", [128, 128], BF16),
    ("kwA", [128, 128], BF16),
    ("kwB", [128, 128], BF16),
    ("vw", [128, 256], BF16),
    ("pwA", [128, 128], BF16),
    ("pwB", [128, 128], BF16),
    ("qwAr", [128, 128], BF16),
    ("qwBr", [128, 128], BF16),
    ("kwAr", [128, 128], BF16),
    ("kwBr", [128, 128], BF16),
    ("cos", [128, N], F32),
    ("sin", [128, N], F32),
    ("cb", [128, 1], F32),
    ("pb", [128, 1], F32),
]


def build_nc(stage: int = 99):
    nc = bacc.Bacc(
        "TRN2",
        target_bir_lowering=False,
        debug=False,
        num_devices=NCORES,
    )
    io = {}
    for name, shape, dt in _SPECS:
        io[name] = nc.dram_tensor(name, shape, dt, kind="ExternalInput").ap()
    io["out"] = nc.dram_tensor("out", [128, N], F32, kind="ExternalOutput").ap()
    with tile.TileContext(nc) as tc:
        build_kernel(tc, io, stage)
    nc.compile()
    return nc


_CACHE = {}


def kernel(x, conv_w, conv_b, qkv_w, proj_w, proj_b):
    if "nc" not in _CACHE:
        _CACHE["nc"] = build_nc()
    nc = _CACHE["nc"]
    host = prep_host(
        np.asarray(conv_w),
        np.asarray(conv_b),
        np.asarray(qkv_w),
        np.asarray(proj_w),
        np.asarray(proj_b),
    )
    x = np.asarray(x, dtype=np.float32)
    xr = x.reshape(NCORES, 128, N)
    xall_bf = xr.astype(NPBF16)
    in_maps = []
    for c in range(NCORES):
        im = dict(host)
        im["xs"] = np.ascontiguousarray(xr[c])
        im["xall"] = np.ascontiguousarray(
            np.roll(xall_bf, -c, axis=0)
        )
        in_maps.append(im)
    res = run_bass_kernel_spmd(nc, in_maps, core_ids=list(range(NCORES)))
    out = np.stack(
        [np.asarray(res.results[c]["out"]).reshape(C, H, W) for c in range(NCORES)]
    )
    return out.astype(np.float32)


# revision 20
# speedup vs baseline: 1.7331x; 1.7331x over previous
"""Trainium2 Bass kernel for AdvancedConvBlock: conv3x3 + batch-stat LN + RoPE
attention with ALiBi + proj + residual, data-parallel over batch on 8 cores.

Self-contained: hardcodes shapes B=8, C=128, H=W=32, heads=8, d=16.
"""

import sys

sys.path.insert(0, "/opt/trn_rl_repo")

import numpy as np
from contextlib import ExitStack

import concourse.bass as bass
import concourse.tile as tile
from concourse import mybir
from concourse import bacc
from concourse.bass_utils import run_bass_kernel_spmd

F32 = mybir.dt.float32
BF16 = mybir.dt.bfloat16
NPBF16 = mybir.dt.np(mybir.dt.bfloat16)

NCORES = 8
C = 128
H = W = 32
N = H * W  # 1024 tokens
NHEADS = 8
D = 16  # head dim
SCALE = D ** (-0.5)
ALIBI_MAX_BIAS = 8.0
EPS = 1e-5
TOTAL = NCORES * N  # stat count 8192

AX = mybir.AxisListType
ALU = mybir.AluOpType
ACT = mybir.ActivationFunctionType


def _alibi_slopes(n: int) -> np.ndarray:
    start = 2.0 ** (-(2.0 ** (-(np.log2(n) - 3.0))))
    return np.array([start * (start ** i) for i in range(n)], dtype=np.float32)


SLOPE8 = _alibi_slopes(NHEADS) * ALIBI_MAX_BIAS  # per-head bias multiplier


# ---------------------------------------------------------------- kernel build
def build_kernel(tc: tile.TileContext, io: dict, stage: int = 99):
    nc = tc.nc
    ctx = ExitStack()
    sb = ctx.enter_context(tc.tile_pool(name="sb", bufs=1))
    work = ctx.enter_context(tc.tile_pool(name="work", bufs=3))
    epool = ctx.enter_context(tc.tile_pool(name="e", bufs=6))
    ps = ctx.enter_context(tc.tile_pool(name="ps", bufs=2, space="PSUM"))
    av_pool = ctx.enter_context(tc.tile_pool(name="av", bufs=2, space="PSUM"))
    dram = ctx.enter_context(tc.tile_pool(name="dram", bufs=1, space="DRAM"))

    # ---- persistent inputs: conv-critical first (sync queue); the rest
    # spread across other engines' DMA queues so nothing blocks conv start.
    cw = sb.tile([128, 9, 128], BF16)
    nc.sync.dma_start(out=cw, in_=io["cwT"])
    cb = sb.tile([128, 1], F32)
    nc.sync.dma_start(out=cb, in_=io["cb"])
    cos = sb.tile([128, N], F32)
    nc.scalar.dma_start(out=cos, in_=io["cos"])
    sin = sb.tile([128, N], F32)
    nc.scalar.dma_start(out=sin, in_=io["sin"])
    qwA = sb.tile([128, 128], BF16)
    nc.scalar.dma_start(out=qwA, in_=io["qwA"])
    qwB = sb.tile([128, 128], BF16)
    nc.scalar.dma_start(out=qwB, in_=io["qwB"])
    kwA = sb.tile([128, 128], BF16)
    nc.scalar.dma_start(out=kwA, in_=io["kwA"])
    kwB = sb.tile([128, 128], BF16)
    nc.scalar.dma_start(out=kwB, in_=io["kwB"])
    qwAr = sb.tile([128, 128], BF16)
    nc.scalar.dma_start(out=qwAr, in_=io["qwAr"])
    qwBr = sb.tile([128, 128], BF16)
    nc.scalar.dma_start(out=qwBr, in_=io["qwBr"])
    kwAr = sb.tile([128, 128], BF16)
    nc.scalar.dma_start(out=kwAr, in_=io["kwAr"])
    kwBr = sb.tile([128, 128], BF16)
    nc.scalar.dma_start(out=kwBr, in_=io["kwBr"])
    vw = sb.tile([128, 256], BF16)
    nc.scalar.dma_start(out=vw, in_=io["vw"])
    pwA = sb.tile([128, 128], BF16)
    nc.gpsimd.dma_start(out=pwA, in_=io["pwA"])
    pwB = sb.tile([128, 128], BF16)
    nc.gpsimd.dma_start(out=pwB, in_=io["pwB"])
    pb = sb.tile([128, 1], F32)
    nc.gpsimd.dma_start(out=pb, in_=io["pb"])
    x_f32 = sb.tile([128, N], F32)
    nc.gpsimd.dma_start(out=x_f32, in_=io["xs"])
    m_sb = sb.tile([128, 8, 1536], BF16)  # alibi decay table per head
    nc.gpsimd.dma_start(out=m_sb, in_=io["m"])

    # ---- conv 3x3 pad 1 on ALL 8 images (own image = slot 0); global batch
    # stats computed locally — no collective needed.
    xbs = []
    for img in range(8):
        xb = sb.tile([128, N], BF16, tag=f"xb{img}", name=f"xb{img}")
        nc.sync.dma_start(out=xb, in_=io["xall"][img])
        xbs.append(xb)
    scol = sb.tile([128, 8], F32)
    sqcol = sb.tile([128, 8], F32)
    y_sb = sb.tile([128, N], F32)
    for img in range(8):
        xpad = work.tile([128, 34, 34], BF16, tag="xpad")
        nc.vector.memset(xpad, 0.0)
        nc.vector.tensor_copy(
            xpad[:, 1:33, 1:33], xbs[img].rearrange("p (h w) -> p h w", h=H)
        )
        conv_ps = ps.tile([128, N], F32, tag="ps")
        conv_v = conv_ps.rearrange("p (h w) -> p h w", h=H)
        for t in range(9):
            dh, dw = t // 3, t % 3
            for hc in range(2):
                nc.tensor.matmul(
                    out=conv_v[:, hc * 16 : hc * 16 + 16, :],
                    lhsT=cw[:, t, :],
                    rhs=xpad[:, dh + hc * 16 : dh + hc * 16 + 16, dw : dw + 32],
                    start=(t == 0),
                    stop=(t == 8),
                )
        nc.vector.tensor_reduce(
            scol[:, img : img + 1], conv_ps, axis=AX.X, op=ALU.add
        )
        sq_dump = work.tile([128, N], F32, tag="sqd")
        nc.scalar.activation(
            sq_dump, conv_ps, ACT.Square, accum_out=sqcol[:, img : img + 1]
        )
        if img == 0:
            nc.vector.tensor_scalar_add(y_sb, conv_ps, cb)

    # global per-channel stats of y = conv + cb over all 8*1024 samples
    s_t = sb.tile([128, 1], F32)
    nc.vector.tensor_reduce(s_t, scol, axis=AX.X, op=ALU.add)
    sq_t = sb.tile([128, 1], F32)
    nc.vector.tensor_reduce(sq_t, sqcol, axis=AX.X, op=ALU.add)
    mean0 = sb.tile([128, 1], F32)
    nc.scalar.mul(mean0, s_t, 1.0 / TOTAL)
    mean = sb.tile([128, 1], F32)
    nc.vector.tensor_add(mean, mean0, cb)
    ex2 = sb.tile([128, 1], F32)
    nc.scalar.mul(ex2, sq_t, 1.0 / TOTAL)
    # ex2 of (conv+cb) = E[conv^2] + cb*(2*mean0 + cb)
    t2m = sb.tile([128, 1], F32)
    nc.vector.tensor_add(t2m, mean0, mean0)
    nc.vector.tensor_add(t2m, t2m, cb)
    nc.vector.tensor_mul(t2m, t2m, cb)
    nc.vector.tensor_add(ex2, ex2, t2m)
    var = sb.tile([128, 1], F32)
    nc.vector.tensor_mul(var, mean, mean)
    nc.vector.tensor_sub(var, ex2, var)
    eps_t = sb.tile([128, 1], F32)
    nc.vector.memset(eps_t, EPS)
    std = sb.tile([128, 1], F32)
    nc.scalar.activation(std, var, ACT.Sqrt, bias=eps_t)
    rstd = sb.tile([128, 1], F32)
    nc.vector.reciprocal(rstd, std)
    nmb = sb.tile([128, 1], F32)
    nc.vector.tensor_mul(nmb, mean, rstd)
    nc.scalar.mul(nmb, nmb, -1.0)
    y_n = sb.tile([128, N], BF16)
    nc.scalar.activation(y_n, y_sb, ACT.Identity, bias=nmb, scale=rstd)
    if stage <= 1:
        dbg = sb.tile([128, N], F32)
        nc.vector.tensor_copy(dbg, y_n)
        nc.sync.dma_start(out=io["out"], in_=dbg)
        ctx.close()
        return

    # ---- qkv with RoPE fused: q' = (W y)*cos + ((P W) y)*sin, packed heads
    def qk_rope(wt, wrt, name):
        p0 = ps.tile([128, N], F32, tag="ps")
        p1 = ps.tile([128, N], F32, tag="ps")
        for c in range(2):
            sl = slice(c * 512, (c + 1) * 512)
            nc.tensor.matmul(
                out=p0[:, sl], lhsT=wt, rhs=y_n[:, sl], start=True, stop=True
            )
            nc.tensor.matmul(
                out=p1[:, sl], lhsT=wrt, rhs=y_n[:, sl], start=True, stop=True
            )
        t1 = work.tile([128, N], F32, tag="ropet1")
        nc.vector.tensor_mul(t1, p0, cos)
        t2 = work.tile([128, N], F32, tag="ropet2")
        nc.vector.tensor_mul(t2, p1, sin)
        out = sb.tile([128, N], BF16, tag=name)
        nc.vector.tensor_add(out, t1, t2)
        return out

    qAr = qk_rope(qwA, qwAr, "qAr")
    qBr = qk_rope(qwB, qwBr, "qBr")
    kAr = qk_rope(kwA, kwAr, "kAr")
    kBr = qk_rope(kwB, kwBr, "kBr")

    # ---- v transposed: vt[j, head, dcol] with a ones column at dcol=16
    vt = sb.tile([128, 8, 8, 32], BF16)  # [j-part, jc, head, 32]
    for jc in range(8):
        vp = ps.tile([128, 256], F32, tag="ps")
        nc.tensor.matmul(
            out=vp,
            lhsT=y_n[:, jc * 128 : (jc + 1) * 128],
            rhs=vw,
            start=True,
            stop=True,
        )
        nc.vector.tensor_copy(vt[:, jc], vp.rearrange("p (h c) -> p h c", c=32))
    nc.vector.memset(vt[:, :, :, 0:1], 1.0)

    if stage <= 2:
        dbg = sb.tile([128, N], F32)
        nc.vector.tensor_copy(dbg, qAr)
        nc.vector.tensor_add(dbg, dbg, kBr)
        nc.sync.dma_start(out=io["out"], in_=dbg)
        ctx.close()
        return

    # ---- attention: transposed scores s[j, i], z-deferred softmax.
    # Per (group, key-chunk jc, query-half ic): 4 packed score MMs into one
    # [128, 2048] psum tile, one exp, one decay-table multiply, 4 AV MMs.
    SKIP_SET = {0: {0, 1, 2}, 1: {0, 1, 2}, 2: {0, 1, 2}, 3: {0, 1, 2},
                4: {0, 1}, 5: {0}, 6: set(), 7: set()}

    def present(h, jc, ic):
        return ic == 0 or jc not in SKIP_SET[h]

    def jc_range(h, ic):
        return [jc for jc in range(8) if present(h, jc, ic)]

    o_pks = []
    for g in range(2):
        q_r = qAr if g == 0 else qBr
        k_r = kAr if g == 0 else kBr
        o_acc = av_pool.tile([128, N], F32)
        pend = []

        def flush_av(n_keep):
            while len(pend) > n_keep:
                e2_, pres_, hp_, jc_, ic_ = pend.pop(0)
                isl_ = slice(ic_ * 512, (ic_ + 1) * 512)
                for hh in pres_:
                    h = 4 * g + hh
                    jr = jc_range(h, ic_)
                    nc.tensor.matmul(
                        out=o_acc[32 * hh : 32 * hh + 32, isl_],
                        lhsT=vt[:, jc_, h, :],
                        rhs=e2_[:, hh - 2 * hp_, :],
                        start=(jc_ == jr[0]),
                        stop=(jc_ == jr[-1]),
                        tile_position=(0, 32 * hh),
                        skip_group_check=True,
                    )

        for jc in range(8):
            for ic in range(2):
                isl = slice(ic * 512, (ic + 1) * 512)
                touches_past = 128 * jc < 512 * (ic + 1)
                off = 512 - 128 * jc + 512 * ic
                for hp in range(2):
                    pres = [
                        hh
                        for hh in (2 * hp, 2 * hp + 1)
                        if present(4 * g + hh, jc, ic)
                    ]
                    if not pres:
                        continue
                    s2 = ps.tile([128, 2, 512], F32, tag="ps")
                    for hh in pres:
                        nc.tensor.matmul(
                            out=s2[:, hh - 2 * hp, :],
                            lhsT=k_r[
                                32 * hh : 32 * hh + 16, jc * 128 : (jc + 1) * 128
                            ],
                            rhs=q_r[32 * hh : 32 * hh + 16, isl],
                            start=True,
                            stop=True,
                            tile_position=(32 * hh, 0),
                        )
                    lo = pres[0] - 2 * hp
                    e2 = epool.tile([128, 2, 512], BF16, tag="e")
                    nc.scalar.activation(e2[:, lo:, :], s2[:, lo:, :], ACT.Exp)
                    if touches_past:
                        nc.vector.tensor_mul(
                            e2[:, lo:, :],
                            e2[:, lo:, :],
                            m_sb[:, 4 * g + pres[0] : 4 * g + 2 * hp + 2, off : off + 512],
                        )
                    pend.append((e2, pres, hp, jc, ic))
                    flush_av(2)
        flush_av(0)
        # normalize: row 32h is Z (DRAM-roundtrip broadcast), rows +1..+16 are o
        o_pk = sb.tile([128, N], BF16, tag=f"opk{g}")
        zsb = work.tile([128, N], F32, tag="zsb")
        nc.vector.tensor_copy(zsb, o_acc)
        zd = dram.tile([4, N], F32, tag="zd")
        nc.sync.dma_start(out=zd, in_=zsb[0:128:32, :])
        zbc = work.tile([128, N], F32, tag="zbc")
        for hh in range(4):
            nc.sync.dma_start(
                out=zbc[32 * hh : 32 * hh + 32, :],
                in_=zd[hh : hh + 1, :].broadcast_to([32, N]),
            )
        rz = work.tile([128, N], F32, tag="rz")
        nc.vector.reciprocal_approx_fast(rz, zbc)
        nc.vector.tensor_mul(o_pk, zsb, rz)
        o_pks.append(o_pk)
    if stage <= 3:
        dbg = sb.tile([128, N], F32)
        nc.vector.tensor_copy(dbg, o_pks[0])
        nc.sync.dma_start(out=io["out"], in_=dbg)
        ctx.close()
        return

    # ---- output proj + bias + residual
    pr_ps = ps.tile([128, N], F32, tag="ps")
    for c in range(2):
        sl = slice(c * 512, (c + 1) * 512)
        nc.tensor.matmul(
            out=pr_ps[:, sl], lhsT=pwA, rhs=o_pks[0][:, sl], start=True, stop=False
        )
        nc.tensor.matmul(
            out=pr_ps[:, sl], lhsT=pwB, rhs=o_pks[1][:, sl], start=False, stop=True
        )
    out_sb = sb.tile([128, N], F32)
    nc.vector.scalar_tensor_tensor(
        out=out_sb,
        in0=pr_ps,
        scalar=pb,
        in1=x_f32,
        op0=ALU.add,
        op1=ALU.add,
    )
    nc.sync.dma_start(out=io["out"], in_=out_sb)
    ctx.close()


# ---------------------------------------------------------------- host side
def prep_host(conv_w, conv_b, qkv_w, proj_w, proj_b):
    """Precompute packed / transposed weight + table arrays shared by all cores."""
    cwT = (
        conv_w.astype(np.float32)
        .transpose(1, 2, 3, 0)
        .reshape(128, 9, 128)
        .astype(NPBF16)
    )
    qw = qkv_w[0:128]  # [128 feat, 128 ci]
    kw = qkv_w[128:256]
    vwm = qkv_w[256:384]

    def pack_qk(wm, scale):
        # lhsT[ci, 32g+r] = wm[16g+r, ci] * scale for r<16 else 0  (two halves)
        outA = np.zeros((128, 128), np.float32)
        outB = np.zeros((128, 128), np.float32)
        for g in range(4):
            for r in range(16):
                outA[:, 32 * g + r] = wm[16 * g + r, :] * scale
                outB[:, 32 * g + r] = wm[16 * (g + 4) + r, :] * scale
        return outA.astype(NPBF16), outB.astype(NPBF16)

    qwA, qwB = pack_qk(qw, SCALE)
    kwA, kwB = pack_qk(kw, 1.0)
    # rotate-half fold: rot(W y) = (P W) y, applied to packed lhsT [ci, m]
    P = np.zeros((128, 128), np.float32)
    for gg in range(4):
        b = 32 * gg
        for r in range(8):
            P[b + r, b + r + 8] = -1.0
            P[b + r + 8, b + r] = 1.0
    def rot(w):
        return (w.astype(np.float32) @ P.T).astype(NPBF16)
    qwAr, qwBr = rot(qwA), rot(qwB)
    kwAr, kwBr = rot(kwA), rot(kwB)
    vw = np.zeros((128, 256), np.float32)
    for h in range(8):
        for d in range(16):
            vw[:, 32 * h + 1 + d] = vwm[16 * h + d, :]
    vw = vw.astype(NPBF16)

    # proj: lhsT[packed_row, c'] = proj_w[c', feat(packed_row)]
    pwA = np.zeros((128, 128), np.float32)
    pwB = np.zeros((128, 128), np.float32)
    for g in range(4):
        for r in range(16):
            pwA[32 * g + 1 + r, :] = proj_w[:, 16 * g + r]
            pwB[32 * g + 1 + r, :] = proj_w[:, 16 * (g + 4) + r]
    pwA = pwA.astype(NPBF16)
    pwB = pwB.astype(NPBF16)

    inv_freq = 1.0 / (10000.0 ** (np.arange(0, D, 2, dtype=np.float32) / D))
    pos = np.arange(N, dtype=np.float32)
    freqs = pos[:, None] * inv_freq[None, :]  # [N, 8]
    cos_t = np.zeros((128, N), np.float32)
    sin_t = np.zeros((128, N), np.float32)
    for g in range(4):
        for r in range(16):
            cos_t[32 * g + r, :] = np.cos(freqs[:, r % 8])
            sin_t[32 * g + r, :] = np.sin(freqs[:, r % 8])

    # alibi decay table tblm[p, h, c] = exp(slope8[h] * min(p - c + 512, 0))
    p_ = np.arange(128, dtype=np.float64)[:, None, None]
    c_ = np.arange(1536, dtype=np.float64)[None, None, :]
    d_ = np.minimum(p_ - c_ + 512.0, 0.0)
    m = np.exp(SLOPE8.astype(np.float64)[None, :, None] * d_).astype(NPBF16)

    return dict(
        cwT=cwT,
        qwA=qwA,
        qwB=qwB,
        kwA=kwA,
        kwB=kwB,
        vw=vw,
        pwA=pwA,
        pwB=pwB,
        qwAr=qwAr,
        qwBr=qwBr,
        kwAr=kwAr,
        kwBr=kwBr,
        cos=cos_t,
        sin=sin_t,
        m=m,
        cb=conv_b.astype(np.float32).reshape(128, 1),
        pb=proj_b.astype(np.float32).reshape(128, 1),
    )


_SPECS = [
    ("xs", [128, N], F32),
    ("xall", [8, 128, N], BF16),
    ("m", [128, 8, 1536], BF16),
    ("cwT", [128, 9, 128], BF16),
    ("qwA", [128, 128], BF16),
    ("qwB", [128, 128], BF16),
    ("kwA", [128, 128], BF16),
    ("kwB", [128, 128], BF16),
    ("vw", [128, 256], BF16),
    ("pwA", [128, 128], BF16),
    ("pwB", [128, 128], BF16),
    ("qwAr", [128, 128], BF16),
    ("qwBr", [128, 128], BF16),
    ("kwAr", [128, 128], BF16),
    ("kwBr", [128, 128], BF16),
    ("cos", [128, N], F32),
    ("sin", [128, N], F32),
    ("cb", [128, 1], F32),
    ("pb", [128, 1], F32),
]


def build_nc(stage: int = 99):
    nc = bacc.Bacc(
        "TRN2",
        target_bir_lowering=False,
        debug=False,
        num_devices=NCORES,
    )
    io = {}
    for name, shape, dt in _SPECS:
        io[name] = nc.dram_tensor(name, shape, dt, kind="ExternalInput").ap()
    io["out"] = nc.dram_tensor("out", [128, N], F32, kind="ExternalOutput").ap()
    with tile.TileContext(nc) as tc:
        build_kernel(tc, io, stage)
    nc.compile()
    return nc


_CACHE = {}


def kernel(x, conv_w, conv_b, qkv_w, proj_w, proj_b):
    if "nc" not in _CACHE:
        _CACHE["nc"] = build_nc()
    nc = _CACHE["nc"]
    host = prep_host(
        np.asarray(conv_w),
        np.asarray(conv_b),
        np.asarray(qkv_w),
        np.asarray(proj_w),
        np.asarray(proj_b),
    )
    x = np.asarray(x, dtype=np.float32)
    xr = x.reshape(NCORES, 128, N)
    xall_bf = xr.astype(NPBF16)
    in_maps = []
    for c in range(NCORES):
        im = dict(host)
        im["xs"] = np.ascontiguousarray(xr[c])
        im["xall"] = np.ascontiguousarray(
            np.roll(xall_bf, -c, axis=0)
        )
        in_maps.append(im)
    res = run_bass_kernel_spmd(nc, in_maps, core_ids=list(range(NCORES)))
    out = np.stack(
        [np.asarray(res.results[c]["out"]).reshape(C, H, W) for c in range(NCORES)]
    )
    return out.astype(np.float32)


# revision 21
# speedup vs baseline: 1.7378x; 1.0027x over previous
"""Trainium2 Bass kernel for AdvancedConvBlock: conv3x3 + batch-stat LN + RoPE
attention with ALiBi + proj + residual, data-parallel over batch on 8 cores.

Self-contained: hardcodes shapes B=8, C=128, H=W=32, heads=8, d=16.
"""

import sys

sys.path.insert(0, "/opt/trn_rl_repo")

import numpy as np
from contextlib import ExitStack

import concourse.bass as bass
import concourse.tile as tile
from concourse import mybir
from concourse import bacc
from concourse.bass_utils import run_bass_kernel_spmd

F32 = mybir.dt.float32
BF16 = mybir.dt.bfloat16
NPBF16 = mybir.dt.np(mybir.dt.bfloat16)

NCORES = 8
C = 128
H = W = 32
N = H * W  # 1024 tokens
NHEADS = 8
D = 16  # head dim
SCALE = D ** (-0.5)
ALIBI_MAX_BIAS = 8.0
EPS = 1e-5
TOTAL = NCORES * N  # stat count 8192

AX = mybir.AxisListType
ALU = mybir.AluOpType
ACT = mybir.ActivationFunctionType


def _alibi_slopes(n: int) -> np.ndarray:
    start = 2.0 ** (-(2.0 ** (-(np.log2(n) - 3.0))))
    return np.array([start * (start ** i) for i in range(n)], dtype=np.float32)


SLOPE8 = _alibi_slopes(NHEADS) * ALIBI_MAX_BIAS  # per-head bias multiplier


# ---------------------------------------------------------------- kernel build
def build_kernel(tc: tile.TileContext, io: dict, stage: int = 99):
    nc = tc.nc
    ctx = ExitStack()
    sb = ctx.enter_context(tc.tile_pool(name="sb", bufs=1))
    work = ctx.enter_context(tc.tile_pool(name="work", bufs=3))
    epool = ctx.enter_context(tc.tile_pool(name="e", bufs=6))
    ps = ctx.enter_context(tc.tile_pool(name="ps", bufs=2, space="PSUM"))
    av_pool = ctx.enter_context(tc.tile_pool(name="av", bufs=2, space="PSUM"))
    dram = ctx.enter_context(tc.tile_pool(name="dram", bufs=1, space="DRAM"))

    # ---- persistent inputs: conv-critical first (sync queue); the rest
    # spread across other engines' DMA queues so nothing blocks conv start.
    cw = sb.tile([128, 9, 128], BF16)
    nc.sync.dma_start(out=cw, in_=io["cwT"])
    cb = sb.tile([128, 1], F32)
    nc.sync.dma_start(out=cb, in_=io["cb"])
    cos = sb.tile([128, N], F32)
    nc.scalar.dma_start(out=cos, in_=io["cos"])
    sin = sb.tile([128, N], F32)
    nc.scalar.dma_start(out=sin, in_=io["sin"])
    qwA = sb.tile([128, 128], BF16)
    nc.scalar.dma_start(out=qwA, in_=io["qwA"])
    qwB = sb.tile([128, 128], BF16)
    nc.scalar.dma_start(out=qwB, in_=io["qwB"])
    kwA = sb.tile([128, 128], BF16)
    nc.scalar.dma_start(out=kwA, in_=io["kwA"])
    kwB = sb.tile([128, 128], BF16)
    nc.scalar.dma_start(out=kwB, in_=io["kwB"])
    qwAr = sb.tile([128, 128], BF16)
    nc.scalar.dma_start(out=qwAr, in_=io["qwAr"])
    qwBr = sb.tile([128, 128], BF16)
    nc.scalar.dma_start(out=qwBr, in_=io["qwBr"])
    kwAr = sb.tile([128, 128], BF16)
    nc.scalar.dma_start(out=kwAr, in_=io["kwAr"])
    kwBr = sb.tile([128, 128], BF16)
    nc.scalar.dma_start(out=kwBr, in_=io["kwBr"])
    vw = sb.tile([128, 256], BF16)
    nc.scalar.dma_start(out=vw, in_=io["vw"])
    pwA = sb.tile([128, 128], BF16)
    nc.gpsimd.dma_start(out=pwA, in_=io["pwA"])
    pwB = sb.tile([128, 128], BF16)
    nc.gpsimd.dma_start(out=pwB, in_=io["pwB"])
    pb = sb.tile([128, 1], F32)
    nc.gpsimd.dma_start(out=pb, in_=io["pb"])
    x_f32 = sb.tile([128, N], F32)
    nc.gpsimd.dma_start(out=x_f32, in_=io["xs"])
    m_sb = sb.tile([128, 8, 1536], BF16)  # alibi decay table per head
    nc.gpsimd.dma_start(out=m_sb, in_=io["m"])

    # ---- conv 3x3 pad 1 on ALL 8 images (own image = slot 0); global batch
    # stats computed locally — no collective needed.
    xbs = []
    for img in range(8):
        xb = sb.tile([128, N], BF16, tag=f"xb{img}", name=f"xb{img}")
        nc.sync.dma_start(out=xb, in_=io["xall"][img])
        xbs.append(xb)
    scol = sb.tile([128, 8], F32)
    sqcol = sb.tile([128, 8], F32)
    y_sb = sb.tile([128, N], F32)
    for img in range(8):
        xpad = work.tile([128, 34, 34], BF16, tag="xpad")
        nc.vector.memset(xpad, 0.0)
        nc.vector.tensor_copy(
            xpad[:, 1:33, 1:33], xbs[img].rearrange("p (h w) -> p h w", h=H)
        )
        conv_ps = ps.tile([128, N], F32, tag="ps")
        conv_v = conv_ps.rearrange("p (h w) -> p h w", h=H)
        for t in range(9):
            dh, dw = t // 3, t % 3
            for hc in range(2):
                nc.tensor.matmul(
                    out=conv_v[:, hc * 16 : hc * 16 + 16, :],
                    lhsT=cw[:, t, :],
                    rhs=xpad[:, dh + hc * 16 : dh + hc * 16 + 16, dw : dw + 32],
                    start=(t == 0),
                    stop=(t == 8),
                )
        nc.vector.tensor_reduce(
            scol[:, img : img + 1], conv_ps, axis=AX.X, op=ALU.add
        )
        sq_dump = work.tile([128, N], F32, tag="sqd")
        nc.scalar.activation(
            sq_dump, conv_ps, ACT.Square, accum_out=sqcol[:, img : img + 1]
        )
        if img == 0:
            nc.vector.tensor_scalar_add(y_sb, conv_ps, cb)

    # keep PE busy (HAM warm) across the stats tail
    warm_ps = ps.tile([128, 512], F32, tag="ps")
    for t in range(8):
        nc.tensor.matmul(
            out=warm_ps,
            lhsT=cw[:, t, :],
            rhs=xbs[0][:, 0:512],
            start=(t == 0),
            stop=(t == 7),
        )
    warm_sb = sb.tile([1, 1], F32)
    nc.vector.tensor_copy(warm_sb, warm_ps[0:1, 0:1])
    warmsink = dram.tile([1, 1], F32)
    nc.sync.dma_start(out=warmsink, in_=warm_sb)

    # global per-channel stats of y = conv + cb over all 8*1024 samples
    s_t = sb.tile([128, 1], F32)
    nc.vector.tensor_reduce(s_t, scol, axis=AX.X, op=ALU.add)
    sq_t = sb.tile([128, 1], F32)
    nc.vector.tensor_reduce(sq_t, sqcol, axis=AX.X, op=ALU.add)
    mean0 = sb.tile([128, 1], F32)
    nc.scalar.mul(mean0, s_t, 1.0 / TOTAL)
    mean = sb.tile([128, 1], F32)
    nc.vector.tensor_add(mean, mean0, cb)
    ex2 = sb.tile([128, 1], F32)
    nc.scalar.mul(ex2, sq_t, 1.0 / TOTAL)
    # ex2 of (conv+cb) = E[conv^2] + cb*(2*mean0 + cb)
    t2m = sb.tile([128, 1], F32)
    nc.vector.tensor_add(t2m, mean0, mean0)
    nc.vector.tensor_add(t2m, t2m, cb)
    nc.vector.tensor_mul(t2m, t2m, cb)
    nc.vector.tensor_add(ex2, ex2, t2m)
    var = sb.tile([128, 1], F32)
    nc.vector.tensor_mul(var, mean, mean)
    nc.vector.tensor_sub(var, ex2, var)
    eps_t = sb.tile([128, 1], F32)
    nc.vector.memset(eps_t, EPS)
    std = sb.tile([128, 1], F32)
    nc.scalar.activation(std, var, ACT.Sqrt, bias=eps_t)
    rstd = sb.tile([128, 1], F32)
    nc.vector.reciprocal(rstd, std)
    nmb = sb.tile([128, 1], F32)
    nc.vector.tensor_mul(nmb, mean, rstd)
    nc.scalar.mul(nmb, nmb, -1.0)
    y_n = sb.tile([128, N], BF16)
    nc.scalar.activation(y_n, y_sb, ACT.Identity, bias=nmb, scale=rstd)
    if stage <= 1:
        dbg = sb.tile([128, N], F32)
        nc.vector.tensor_copy(dbg, y_n)
        nc.sync.dma_start(out=io["out"], in_=dbg)
        ctx.close()
        return

    # ---- qkv with RoPE fused: q' = (W y)*cos + ((P W) y)*sin, packed heads
    def qk_rope(wt, wrt, name):
        p0 = ps.tile([128, N], F32, tag="ps")
        p1 = ps.tile([128, N], F32, tag="ps")
        for c in range(2):
            sl = slice(c * 512, (c + 1) * 512)
            nc.tensor.matmul(
                out=p0[:, sl], lhsT=wt, rhs=y_n[:, sl], start=True, stop=True
            )
            nc.tensor.matmul(
                out=p1[:, sl], lhsT=wrt, rhs=y_n[:, sl], start=True, stop=True
            )
        t1 = work.tile([128, N], F32, tag="ropet1")
        nc.vector.tensor_mul(t1, p0, cos)
        t2 = work.tile([128, N], F32, tag="ropet2")
        nc.vector.tensor_mul(t2, p1, sin)
        out = sb.tile([128, N], BF16, tag=name)
        nc.vector.tensor_add(out, t1, t2)
        return out

    qAr = qk_rope(qwA, qwAr, "qAr")
    qBr = qk_rope(qwB, qwBr, "qBr")
    kAr = qk_rope(kwA, kwAr, "kAr")
    kBr = qk_rope(kwB, kwBr, "kBr")

    # ---- v transposed: vt[j, head, dcol] with a ones column at dcol=16
    vt = sb.tile([128, 8, 8, 32], BF16)  # [j-part, jc, head, 32]
    for jc in range(8):
        vp = ps.tile([128, 256], F32, tag="ps")
        nc.tensor.matmul(
            out=vp,
            lhsT=y_n[:, jc * 128 : (jc + 1) * 128],
            rhs=vw,
            start=True,
            stop=True,
        )
        nc.vector.tensor_copy(vt[:, jc], vp.rearrange("p (h c) -> p h c", c=32))
    nc.vector.memset(vt[:, :, :, 0:1], 1.0)

    if stage <= 2:
        dbg = sb.tile([128, N], F32)
        nc.vector.tensor_copy(dbg, qAr)
        nc.vector.tensor_add(dbg, dbg, kBr)
        nc.sync.dma_start(out=io["out"], in_=dbg)
        ctx.close()
        return

    # ---- attention: transposed scores s[j, i], z-deferred softmax.
    # Per (group, key-chunk jc, query-half ic): 4 packed score MMs into one
    # [128, 2048] psum tile, one exp, one decay-table multiply, 4 AV MMs.
    SKIP_SET = {0: {0, 1, 2}, 1: {0, 1, 2}, 2: {0, 1, 2}, 3: {0, 1, 2},
                4: {0, 1}, 5: {0}, 6: set(), 7: set()}

    def present(h, jc, ic):
        return ic == 0 or jc not in SKIP_SET[h]

    def jc_range(h, ic):
        return [jc for jc in range(8) if present(h, jc, ic)]

    o_pks = []
    for g in range(2):
        q_r = qAr if g == 0 else qBr
        k_r = kAr if g == 0 else kBr
        o_acc = av_pool.tile([128, N], F32)
        o_pk = sb.tile([128, N], BF16, tag=f"opk{g}", name=f"opk{g}")
        pend = []

        def flush_av(n_keep):
            while len(pend) > n_keep:
                e2_, pres_, hp_, jc_, ic_ = pend.pop(0)
                isl_ = slice(ic_ * 512, (ic_ + 1) * 512)
                for hh in pres_:
                    h = 4 * g + hh
                    jr = jc_range(h, ic_)
                    nc.tensor.matmul(
                        out=o_acc[32 * hh : 32 * hh + 32, isl_],
                        lhsT=vt[:, jc_, h, :],
                        rhs=e2_[:, hh - 2 * hp_, :],
                        start=(jc_ == jr[0]),
                        stop=(jc_ == jr[-1]),
                        tile_position=(0, 32 * hh),
                        skip_group_check=True,
                    )

        def divide_half(ic):
            # Z is row 32h of o_acc; broadcast via DRAM roundtrip, then
            # o_pk = o * (1/Z) for this query-half.
            isl_ = slice(ic * 512, (ic + 1) * 512)
            zsb = work.tile([128, 512], F32, tag="zsb")
            nc.vector.tensor_copy(zsb, o_acc[:, isl_])
            zd = dram.tile([4, 512], F32, tag="zd")
            nc.sync.dma_start(out=zd, in_=zsb[0:128:32, :])
            zbc = work.tile([128, 512], F32, tag="zbc")
            for hh in range(4):
                nc.sync.dma_start(
                    out=zbc[32 * hh : 32 * hh + 32, :],
                    in_=zd[hh : hh + 1, :].broadcast_to([32, 512]),
                )
            rz = work.tile([128, 512], F32, tag="rz")
            nc.vector.reciprocal_approx_fast(rz, zbc)
            nc.vector.tensor_mul(o_pk[:, isl_], zsb, rz)

        for ic in range(2):
            for jc in range(8):
                isl = slice(ic * 512, (ic + 1) * 512)
                touches_past = 128 * jc < 512 * (ic + 1)
                off = 512 - 128 * jc + 512 * ic
                for hp in range(2):
                    pres = [
                        hh
                        for hh in (2 * hp, 2 * hp + 1)
                        if present(4 * g + hh, jc, ic)
                    ]
                    if not pres:
                        continue
                    s2 = ps.tile([128, 2, 512], F32, tag="ps")
                    for hh in pres:
                        nc.tensor.matmul(
                            out=s2[:, hh - 2 * hp, :],
                            lhsT=k_r[
                                32 * hh : 32 * hh + 16, jc * 128 : (jc + 1) * 128
                            ],
                            rhs=q_r[32 * hh : 32 * hh + 16, isl],
                            start=True,
                            stop=True,
                            tile_position=(32 * hh, 0),
                        )
                    lo = pres[0] - 2 * hp
                    e2 = epool.tile([128, 2, 512], BF16, tag="e")
                    nc.scalar.activation(e2[:, lo:, :], s2[:, lo:, :], ACT.Exp)
                    if touches_past:
                        nc.vector.tensor_mul(
                            e2[:, lo:, :],
                            e2[:, lo:, :],
                            m_sb[:, 4 * g + pres[0] : 4 * g + 2 * hp + 2, off : off + 512],
                        )
                    pend.append((e2, pres, hp, jc, ic))
                    flush_av(2)
            if ic == 0:
                flush_av(0)
                divide_half(0)
        flush_av(0)
        divide_half(1)
        o_pks.append(o_pk)
    if stage <= 3:
        dbg = sb.tile([128, N], F32)
        nc.vector.tensor_copy(dbg, o_pks[0])
        nc.sync.dma_start(out=io["out"], in_=dbg)
        ctx.close()
        return

    # ---- output proj + bias + residual
    pr_ps = ps.tile([128, N], F32, tag="ps")
    for c in range(2):
        sl = slice(c * 512, (c + 1) * 512)
        nc.tensor.matmul(
            out=pr_ps[:, sl], lhsT=pwA, rhs=o_pks[0][:, sl], start=True, stop=False
        )
        nc.tensor.matmul(
            out=pr_ps[:, sl], lhsT=pwB, rhs=o_pks[1][:, sl], start=False, stop=True
        )
    out_sb = sb.tile([128, N], F32)
    nc.vector.scalar_tensor_tensor(
        out=out_sb,
        in0=pr_ps,
        scalar=pb,
        in1=x_f32,
        op0=ALU.add,
        op1=ALU.add,
    )
    nc.sync.dma_start(out=io["out"], in_=out_sb)
    ctx.close()


# ---------------------------------------------------------------- host side
def prep_host(conv_w, conv_b, qkv_w, proj_w, proj_b):
    """Precompute packed / transposed weight + table arrays shared by all cores."""
    cwT = (
        conv_w.astype(np.float32)
        .transpose(1, 2, 3, 0)
        .reshape(128, 9, 128)
        .astype(NPBF16)
    )
    qw = qkv_w[0:128]  # [128 feat, 128 ci]
    kw = qkv_w[128:256]
    vwm = qkv_w[256:384]

    def pack_qk(wm, scale):
        # lhsT[ci, 32g+r] = wm[16g+r, ci] * scale for r<16 else 0  (two halves)
        outA = np.zeros((128, 128), np.float32)
        outB = np.zeros((128, 128), np.float32)
        for g in range(4):
            for r in range(16):
                outA[:, 32 * g + r] = wm[16 * g + r, :] * scale
                outB[:, 32 * g + r] = wm[16 * (g + 4) + r, :] * scale
        return outA.astype(NPBF16), outB.astype(NPBF16)

    qwA, qwB = pack_qk(qw, SCALE)
    kwA, kwB = pack_qk(kw, 1.0)
    # rotate-half fold: rot(W y) = (P W) y, applied to packed lhsT [ci, m]
    P = np.zeros((128, 128), np.float32)
    for gg in range(4):
        b = 32 * gg
        for r in range(8):
            P[b + r, b + r + 8] = -1.0
            P[b + r + 8, b + r] = 1.0
    def rot(w):
        return (w.astype(np.float32) @ P.T).astype(NPBF16)
    qwAr, qwBr = rot(qwA), rot(qwB)
    kwAr, kwBr = rot(kwA), rot(kwB)
    vw = np.zeros((128, 256), np.float32)
    for h in range(8):
        for d in range(16):
            vw[:, 32 * h + 1 + d] = vwm[16 * h + d, :]
    vw = vw.astype(NPBF16)

    # proj: lhsT[packed_row, c'] = proj_w[c', feat(packed_row)]
    pwA = np.zeros((128, 128), np.float32)
    pwB = np.zeros((128, 128), np.float32)
    for g in range(4):
        for r in range(16):
            pwA[32 * g + 1 + r, :] = proj_w[:, 16 * g + r]
            pwB[32 * g + 1 + r, :] = proj_w[:, 16 * (g + 4) + r]
    pwA = pwA.astype(NPBF16)
    pwB = pwB.astype(NPBF16)

    inv_freq = 1.0 / (10000.0 ** (np.arange(0, D, 2, dtype=np.float32) / D))
    pos = np.arange(N, dtype=np.float32)
    freqs = pos[:, None] * inv_freq[None, :]  # [N, 8]
    cos_t = np.zeros((128, N), np.float32)
    sin_t = np.zeros((128, N), np.float32)
    for g in range(4):
        for r in range(16):
            cos_t[32 * g + r, :] = np.cos(freqs[:, r % 8])
            sin_t[32 * g + r, :] = np.sin(freqs[:, r % 8])

    # alibi decay table tblm[p, h, c] = exp(slope8[h] * min(p - c + 512, 0))
    p_ = np.arange(128, dtype=np.float64)[:, None, None]
    c_ = np.arange(1536, dtype=np.float64)[None, None, :]
    d_ = np.minimum(p_ - c_ + 512.0, 0.0)
    m = np.exp(SLOPE8.astype(np.float64)[None, :, None] * d_).astype(NPBF16)

    return dict(
        cwT=cwT,
        qwA=qwA,
        qwB=qwB,
        kwA=kwA,
        kwB=kwB,
        vw=vw,
        pwA=pwA,
        pwB=pwB,
        qwAr=qwAr,
        qwBr=qwBr,
        kwAr=kwAr,
        kwBr=kwBr,
        cos=cos_t,
        sin=sin_t,
        m=m,
        cb=conv_b.astype(np.float32).reshape(128, 1),
        pb=proj_b.astype(np.float32).reshape(128, 1),
    )


_SPECS = [
    ("xs", [128, N], F32),
    ("xall", [8, 128, N], BF16),
    ("m", [128, 8, 1536], BF16),
    ("cwT", [128, 9, 128], BF16),
    ("qwA", [128, 128], BF16),
    ("qwB", [128, 128], BF16),
    ("kwA", [128, 128], BF16),
    ("kwB", [128, 128], BF16),
    ("vw", [128, 256], BF16),
    ("pwA", [128, 128], BF16),
    ("pwB", [128, 128], BF16),
    ("qwAr", [128, 128], BF16),
    ("qwBr", [128, 128], BF16),
    ("kwAr", [128, 128], BF16),
    ("kwBr", [128, 128], BF16),
    ("cos", [128, N], F32),
    ("sin", [128, N], F32),
    ("cb", [128, 1], F32),
    ("pb", [128, 1], F32),
]


def build_nc(stage: int = 99):
    nc = bacc.Bacc(
        "TRN2",
        target_bir_lowering=False,
        debug=False,
        num_devices=NCORES,
    )
    io = {}
    for name, shape, dt in _SPECS:
        io[name] = nc.dram_tensor(name, shape, dt, kind="ExternalInput").ap()
    io["out"] = nc.dram_tensor("out", [128, N], F32, kind="ExternalOutput").ap()
    with tile.TileContext(nc) as tc:
        build_kernel(tc, io, stage)
    nc.compile()
    return nc


_CACHE = {}


def kernel(x, conv_w, conv_b, qkv_w, proj_w, proj_b):
    if "nc" not in _CACHE:
        _CACHE["nc"] = build_nc()
    nc = _CACHE["nc"]
    host = prep_host(
        np.asarray(conv_w),
        np.asarray(conv_b),
        np.asarray(qkv_w),
        np.asarray(proj_w),
        np.asarray(proj_b),
    )
    x = np.asarray(x, dtype=np.float32)
    xr = x.reshape(NCORES, 128, N)
    xall_bf = xr.astype(NPBF16)
    in_maps = []
    for c in range(NCORES):
        im = dict(host)
        im["xs"] = np.ascontiguousarray(xr[c])
        im["xall"] = np.ascontiguousarray(
            np.roll(xall_bf, -c, axis=0)
        )
        in_maps.append(im)
    res = run_bass_kernel_spmd(nc, in_maps, core_ids=list(range(NCORES)))
    out = np.stack(
        [np.asarray(res.results[c]["out"]).reshape(C, H, W) for c in range(NCORES)]
    )
    return out.astype(np.float32)
